# revision 16
# baseline (speedup 1.0000x reference)
"""Trainium2 Bass kernel for CustomizeLSTMCell (fused 4-matmul LSTM-like cell).

Math (per token row x of N=100000, H=150):
    pre    = s_in @ W_in + s_out @ W_out + h_in @ U_in + h_out @ U_out
    gate   = sigmoid(pre)
    cell   = gate * last_c + gate * gate = gate * (last_c + gate)
    hidden = gate * tanh(cell)
returns (hidden, cell)

Strategy: data-parallel over tokens across 8 cores (12500 rows/core, padded
to 12544 = 14 * 896). Everything runs feature-major (transposed) on chip:
host packs the four activation tensors as XT[600, 12544] and last_c as
cT[150, 12544] per core; weights concatenate to Wcat[600, 150] and stay
SBUF-resident as the stationary matmul operand. Per 448-token tile the PE
computes preT[150, 448] = Wcat.T @ XT-slice as 2 M-halves (128+22 rows) x 5
K-chunks of 120, with fp16 operands (1 cycle/row, half the X read traffic, ~2^-11 rounding).
ACT does sigmoid/tanh, DVE the elementwise adds/muls; outputs store back
feature-major and the host transposes them back.
"""

import numpy as np

N_TOKENS = 100000
UNITS = 150
N_CORES = 8
ROWS_PER_CORE = N_TOKENS // N_CORES  # 12500
TOK = 448                            # tokens per matmul free dim (>=256)
TOKS_PER_MACRO = 2
MACRO = TOK * TOKS_PER_MACRO         # 896
ROWS_PAD = 12544                     # 14 * 896
N_MACROS = ROWS_PAD // MACRO         # 14
KDIM = 4 * UNITS                     # 600
KCHUNK = 120
N_KCHUNKS = KDIM // KCHUNK           # 5
M0 = 128                             # first output-feature half
M1 = UNITS - M0                      # 22

_CACHE = {}


def _build_bass():
    import concourse.bacc as bacc
    import concourse.mybir as mybir
    import concourse.tile as tile

    fp32 = mybir.dt.float32
    mmdt = mybir.dt.float16
    nc = bacc.Bacc("TRN2", target_bir_lowering=False, debug=False,
                   num_devices=N_CORES)

    xT = nc.dram_tensor("xT", [KDIM, ROWS_PAD], mmdt, kind="ExternalInput").ap()
    cT = nc.dram_tensor("cT", [UNITS, ROWS_PAD], mmdt, kind="ExternalInput").ap()
    w = nc.dram_tensor("w", [KDIM, UNITS], mmdt, kind="ExternalInput").ap()
    hT_out = nc.dram_tensor("hT_out", [UNITS, ROWS_PAD], mmdt,
                            kind="ExternalOutput").ap()
    cT_out = nc.dram_tensor("cT_out", [UNITS, ROWS_PAD], mmdt,
                            kind="ExternalOutput").ap()

    AF = mybir.ActivationFunctionType

    # [600, T] viewed as [120, 5, T]
    xT_r = xT.rearrange("(k p) t -> p k t", p=KCHUNK)
    w_r = w.rearrange("(k p) d -> p k d", p=KCHUNK)

    with tile.TileContext(nc) as tc:
        with (
            tc.tile_pool(name="wpool", bufs=1) as wpool,
            tc.tile_pool(name="xpool", bufs=3) as xpool,
            tc.tile_pool(name="cpool", bufs=3) as cpool,
            tc.tile_pool(name="opool", bufs=3) as opool,
            tc.tile_pool(name="small", bufs=3) as small,
            tc.tile_pool(name="psum", bufs=3, space="PSUM") as psum_pool,
        ):
            w_tile = wpool.tile([KCHUNK, N_KCHUNKS, UNITS], mmdt)
            nc.sync.dma_start(w_tile[:, :, :], w_r[:, :, :])

            for m in range(N_MACROS):
                lo, hi = m * MACRO, (m + 1) * MACRO
                x_tile = xpool.tile([KCHUNK, N_KCHUNKS, MACRO], mmdt)
                nc.sync.dma_start(x_tile[:, :, :], xT_r[:, :, lo:hi])
                c0 = cpool.tile([M0, MACRO], mmdt)
                nc.sync.dma_start(c0[:, :], cT[0:M0, lo:hi])
                c1 = cpool.tile([M1, MACRO], mmdt, tag="c1")
                nc.sync.dma_start(c1[:, :], cT[M0:UNITS, lo:hi])

                h0 = opool.tile([M0, MACRO], mmdt, tag="h0")
                h1 = opool.tile([M1, MACRO], mmdt, tag="h1")
                cell0 = opool.tile([M0, MACRO], mmdt, tag="cell0")
                cell1 = opool.tile([M1, MACRO], mmdt, tag="cell1")
                gate0 = small.tile([M0, MACRO], mmdt, tag="gate0")
                gate1 = small.tile([M1, MACRO], mmdt, tag="gate1")

                for t in range(TOKS_PER_MACRO):
                    ts = slice(t * TOK, (t + 1) * TOK)
                    for (mi, mp, msl, gatet, ct, ht, cellt) in (
                        (0, M0, slice(0, M0), gate0, c0, h0, cell0),
                        (1, M1, slice(M0, UNITS), gate1, c1, h1, cell1),
                    ):
                        pre = psum_pool.tile([mp, TOK], fp32, tag=f"pre{mi}")
                        for k in range(N_KCHUNKS):
                            nc.tensor.matmul(
                                pre[:, :],
                                lhsT=w_tile[:, k, msl],
                                rhs=x_tile[:, k, ts],
                                start=(k == 0),
                                stop=(k == N_KCHUNKS - 1),
                            )
                        nc.scalar.activation(gatet[:, ts], pre[:, :], AF.Sigmoid)
                        nc.vector.tensor_add(cellt[:, ts], ct[:, ts], gatet[:, ts])
                        nc.vector.tensor_mul(cellt[:, ts], gatet[:, ts], cellt[:, ts])
                        nc.scalar.activation(ht[:, ts], cellt[:, ts], AF.Tanh)
                        nc.vector.tensor_mul(ht[:, ts], gatet[:, ts], ht[:, ts])

                # Outputs ride the ACT HWDGE ring: HWDGE is FIFO per issuing
                # engine, so putting stores on SP would head-of-line block the
                # next macro's input loads behind this macro's compute.
                if m < N_MACROS - 1:
                    nc.scalar.dma_start(hT_out[0:M0, lo:hi], h0[:, :])
                    nc.scalar.dma_start(hT_out[M0:UNITS, lo:hi], h1[:, :])
                    nc.scalar.dma_start(cT_out[0:M0, lo:hi], cell0[:, :])
                    nc.scalar.dma_start(cT_out[M0:UNITS, lo:hi], cell1[:, :])
                else:
                    for t in range(TOKS_PER_MACRO):
                        tl, th_ = lo + t * TOK, lo + (t + 1) * TOK
                        tsl = slice(t * TOK, (t + 1) * TOK)
                        nc.scalar.dma_start(hT_out[0:M0, tl:th_], h0[:, tsl])
                        nc.scalar.dma_start(hT_out[M0:UNITS, tl:th_], h1[:, tsl])
                        nc.scalar.dma_start(cT_out[0:M0, tl:th_], cell0[:, tsl])
                        nc.scalar.dma_start(cT_out[M0:UNITS, tl:th_], cell1[:, tsl])

    nc.compile()
    return nc


def _get_nc():
    if "nc" not in _CACHE:
        _CACHE["nc"] = _build_bass()
    return _CACHE["nc"]


def kernel(s_in, s_out, h_in, h_out, last_c,
           w_in_input, w_out_input, u_in_input, u_out_input):
    from concourse.bass_utils import run_bass_kernel_spmd

    nc = _get_nc()

    bf16 = np.float16

    wcat = np.ascontiguousarray(
        np.concatenate([w_in_input, w_out_input, u_in_input, u_out_input],
                       axis=0).astype(np.float32)).astype(bf16)

    in_maps = []
    for c in range(N_CORES):
        rows = slice(c * ROWS_PER_CORE, (c + 1) * ROWS_PER_CORE)
        xT = np.zeros((KDIM, ROWS_PAD), dtype=bf16)
        for j, X in enumerate((s_in, s_out, h_in, h_out)):
            xT[j * UNITS:(j + 1) * UNITS, :ROWS_PER_CORE] = \
                np.asarray(X[rows]).T.astype(bf16)
        cTp = np.zeros((UNITS, ROWS_PAD), dtype=np.float16)
        cTp[:, :ROWS_PER_CORE] = np.asarray(last_c[rows]).T.astype(np.float16)
        in_maps.append({"xT": xT, "cT": cTp, "w": wcat})

    res = run_bass_kernel_spmd(nc, in_maps, core_ids=list(range(N_CORES)))

    hidden = np.concatenate(
        [res.results[c]["hT_out"][:, :ROWS_PER_CORE].T for c in range(N_CORES)],
        axis=0).astype(np.float32)
    cell = np.concatenate(
        [res.results[c]["cT_out"][:, :ROWS_PER_CORE].T for c in range(N_CORES)],
        axis=0).astype(np.float32)
    return np.ascontiguousarray(hidden), np.ascontiguousarray(cell)
